# revision 1
# baseline (speedup 1.0000x reference)
"""Trainium2 Bass kernel for nn_NeuroManifoldBlock (dense transformer block with
FitzHugh-Nagumo-evolved attention scores), SPMD across 8 NeuronCores.

Sharding: cores 0-3 -> batch 0, cores 4-7 -> batch 1. Within a batch group of
4 cores: the sdr projection is feature-sharded and joined by a bf16 on-chip
AllGather; attention is head-sharded (4 heads/core); MLP + output are
token-sharded, fed by two pipelined bf16 ReduceScatters (token halves, so
half A's residual/LN2 overlaps half B's collective) that sum the per-head
out-projection partials. Collective groups: [[0-3],[4-7]].

Key algorithmic choices:
 - All matmuls in bf16 with fp32 PSUM accumulation.
 - The 4-step FHN IMEX integration is an elementwise function g(s) of the
   pre-softmax scores only (v0=s, w0=0); evaluated as a degree-10 polynomial
   fit of g on the reachable score range (|s| < 3.35) in 4 fused passes:
   free ACT-Relu clamp on the PSUM->SBUF copy, ACT Square (quadratic init),
   then 3 custom-DVE Horner3 ops registered at runtime.
 - LayerNorm 1 folds into the QKV projection as a rank-1 matmul correction
   (K=1 accumulate of colsum(W) x mean-row) plus a per-token rsqrt scale,
   so normalized activations are never materialized; LN gains/biases fold
   into the adjacent weights on the host.
 - Softmax without max-subtraction (logits bounded by construction); the
   denominator comes from ones-vector matmuls and divides ctx afterwards;
   causal masking is a bf16 multiply with 4 host-built tile masks applied
   only to partial diagonal tiles.
"""

import numpy as np
import ml_dtypes

from concourse import bass, bacc, tile
import concourse.mybir as mybir
from concourse.bass_utils import run_bass_kernel_spmd

# ---------------------------------------------------------------- constants
B, T, SDR, D, H, DH = 2, 1024, 2048, 1024, 16, 64
FFN = 2730
FFN_PAD = 2816          # 22 * 128
N_CORES = 8
GROUP = 4               # cores per batch
HPC = 4                 # heads per core
TPC = 256               # tokens per core
DT_, FA, FB, FTAU, FTH = 0.1, 0.7, 0.8, 12.5, 0.5
EPS = 1e-5
CLAMP = 3.35
POLY_DEG = 10

F32 = mybir.dt.float32
BF16 = mybir.dt.bfloat16
_bfd = ml_dtypes.bfloat16


def _bf16(x):
    return np.ascontiguousarray(np.asarray(x, np.float32).astype(_bfd))


def _f32(x):
    return np.ascontiguousarray(np.asarray(x, np.float32))


# ------------------------------------------------------- FHN poly (host fit)
def _fhn_g(s):
    s = np.asarray(s, np.float64)
    v = s.copy()
    w = np.zeros_like(s)
    wd = 1.0 + DT_ * FB / FTAU
    for _ in range(4):
        v = v + DT_ * (v - v ** 3 / 3.0 - w + s)
        w = (w + DT_ * (v + FA) / FTAU) / wd
    return v - FTH


def _fit_poly():
    # poly in u = relu(s_raw + 8*CLAMP), s_raw = unscaled scores (q.k)
    xs = np.linspace(0.0, 2 * 8 * CLAMP, 400001)
    g = _fhn_g(xs / 8.0 - CLAMP)
    c = np.polynomial.chebyshev.Chebyshev.fit(xs, g, POLY_DEG)
    return c.convert(kind=np.polynomial.Polynomial).coef[::-1].astype(np.float64)


POLY = _fit_poly()
_c0, _c1, _c2 = POLY[0], POLY[1], POLY[2]
SQ_SIGN = 1.0 if _c0 > 0 else -1.0
_SW = np.sqrt(abs(_c0))
SQ_SCALE = float(_SW)
SQ_BIAS = float(_SW * (_c1 / (2 * _c0)))
SQ_GAMMA = float(_c2 - _c1 ** 2 / (4 * _c0))
HC = [float(c) for c in POLY[3:]]
assert len(HC) == 8


# ------------------------------------------------- custom DVE ops (runtime)
def _register_custom_ops():
    from concourse import dve_ops as DO
    from concourse.dve_spec import Spec, Src0, Src1, C0, C1, C2, lower
    from concourse.dve_uop import DveOpSpec

    defs = {
        "ANT_TT_MULT_ADDC": Spec(
            body=Src0 * Src1 + C0,
            reference=lambda in0, in1, s0, s1, imm2: (
                in0.astype(np.float32) * in1 + s0),
        ),
        "ANT_TT_ADDC_MULT": Spec(
            body=(Src0 + C0) * Src1,
            reference=lambda in0, in1, s0, s1, imm2: (
                (in0.astype(np.float32) + s0) * in1),
        ),
        "ANT_MUL_C_ADD_T": Spec(
            body=Src0 * C0 + Src1,
            reference=lambda in0, in1, s0, s1, imm2: (
                in0.astype(np.float32) * s0 + in1),
        ),
        "ANT_H3_NEG": Spec(
            body=((C0 - Src0) * Src1 + C1) * Src1 + C2,
            reference=lambda in0, in1, s0, s1, imm2: (
                ((s0 - in0.astype(np.float32)) * in1 + s1) * in1 + imm2),
        ),
        "ANT_H3_POS": Spec(
            body=((C0 + Src0) * Src1 + C1) * Src1 + C2,
            reference=lambda in0, in1, s0, s1, imm2: (
                ((s0 + in0.astype(np.float32)) * in1 + s1) * in1 + imm2),
        ),
        "ANT_H3": Spec(
            body=((Src0 * Src1 + C0) * Src1 + C1) * Src1 + C2,
            reference=lambda in0, in1, s0, s1, imm2: (
                ((in0.astype(np.float32) * in1 + s0) * in1 + s1) * in1 + imm2),
        ),
        "ANT_H2": Spec(
            body=(Src0 * Src1 + C0) * Src1 + C1,
            reference=lambda in0, in1, s0, s1, imm2: (
                (in0.astype(np.float32) * in1 + s0) * in1 + s1),
        ),
    }
    existing = {op.name for op in DO.OPS}
    for name, spec in defs.items():
        if name in existing:
            continue
        row = max(DO._SUB_OPCODE_FOR_NAME.values()) + 1
        assert row < 0x20
        DO._SUB_OPCODE_FOR_NAME[name] = row
        shas = {}
        for ver in ("v3", "v4"):
            try:
                shas[ver] = DveOpSpec(
                    name=name, opcode=row, uops=lower(spec, ver=ver),
                    rd1_en=True).sha(ver)
            except Exception:
                pass
        op = DO.DveOp(name, spec, subdim=False, uops_sha=shas)
        DO.OPS.append(op)
        DO.CUSTOM_DVE_SPECS[name] = spec
    return {op.name: op for op in DO.OPS}


_OPS = _register_custom_ops()


# ----------------------------------------------------------- graph builder
def build_graph(debug=False, single=False):
    nc = bacc.Bacc("TRN2", target_bir_lowering=False, debug=False,
                   num_devices=(1 if single else N_CORES))

    # const APs for float biases used by non-Copy activations
    for val in (float(EPS), float(8.0 * CLAMP), float(SQ_BIAS)):
        if (F32, val) not in nc.const_aps.aps:
            t_ = nc.alloc_sbuf_tensor(
                f"const-f32-{abs(hash(val)) % 10**8}", [128, 1], F32)
            nc.gpsimd.memset(t_.ap(), val)
            nc.const_aps.aps[(F32, val)] = t_.ap()
    nc.all_engine_barrier()

    def din(name, shape, dtype):
        return nc.dram_tensor(name, list(shape), dtype, kind="ExternalInput").ap()

    sdrT = din("sdrT", (SDR, T), BF16)
    sdrT_tok = din("sdrT_tok", (SDR, TPC), BF16)
    wsdr = din("wsdr", (SDR, D), BF16)
    wsdr_my = din("wsdr_my", (SDR, 256), BF16)
    sdrb_my = din("sdrb_my", (256, 1), F32)
    sdr_b_row = din("sdr_b_row", (1, D), F32)
    wqk = din("wqk", (D, 512), BF16)
    qk_bias = din("qk_bias", (512, 1), F32)
    qk_csum = din("qk_csum", (1, 512), F32)
    wv = din("wv", (D, 256), BF16)
    v_csum = din("v_csum", (1, 256), F32)
    v_bias_bc = din("v_bias_bc", (128, 256), F32)
    wout = din("wout", (256, D), BF16)
    bout_row = din("bout_row", (1, D), F32)
    masks = din("masks", (4 * 128, 512), BF16)
    ident = din("ident", (128, 128), BF16)
    identf = din("identf", (128, 128), F32)
    wg_t = din("wg_t", (11 * 8 * 128, 256), BF16)
    wu_t = din("wu_t", (11 * 8 * 128, 256), BF16)
    g_bias = din("g_bias", (FFN_PAD, 1), F32)
    u_bias = din("u_bias", (FFN_PAD, 1), F32)
    wd_t = din("wd_t", (22 * 2 * 128, 512), BF16)

    out_ap = nc.dram_tensor("out_slice", [TPC, D], F32,
                            kind="ExternalOutput").ap()
    dbg = {}
    if debug:
        def dout(name, shape, dtype=F32):
            dbg[name] = nc.dram_tensor(name, list(shape), dtype,
                                       kind="ExternalOutput").ap()
        dout("dbg_xtok", (TPC, D))
        dout("dbg_q", (4 * DH, T))
        dout("dbg_k", (4 * DH, T))
        dout("dbg_v", (T, 4 * DH))
        dout("dbg_ctx", (4 * DH, T))
        dout("dbg_attn", (T, D), BF16)
        dout("dbg_h2", (TPC, D))
        dout("dbg_su", (FFN_PAD, TPC))

    TT = 2         # 512-token column tiles
    NDT = 8        # 128-feature tiles of D
    NKK = 16       # 128-row chunks of SDR

    from concourse.dve_ops import OPS as _ops_list
    OP = {o.name: o for o in _ops_list}
    H3S = OP["ANT_H3_POS"] if SQ_SIGN > 0 else OP["ANT_H3_NEG"]
    AF = mybir.ActivationFunctionType
    ALU = mybir.AluOpType

    with tile.TileContext(nc) as tc:
        pp = tc.alloc_tile_pool(name="persist", bufs=1)
        psp = tc.alloc_tile_pool(name="psum", bufs=1, space="PSUM")
        dram = tc.alloc_tile_pool(name="dram", bufs=1, space="DRAM")
        mw = tc.alloc_tile_pool(name="mlpw", bufs=1)
        sp2 = tc.alloc_tile_pool(name="sdrp2", bufs=1)
        sp = tc.alloc_tile_pool(name="sdrp", bufs=1)

        # ---------------- persistent small tiles ----------------
        ones_col = pp.tile([128, 1], BF16, name="ones_col")
        nc.vector.memset(ones_col[:], 1.0)
        ones_row_f = pp.tile([1, 128], F32, name="ones_row_f")
        nc.vector.memset(ones_row_f[:], 1.0)
        ident_sb = pp.tile([128, 128], BF16, name="ident_sb")
        identf_sb = pp.tile([128, 128], F32, name="identf_sb")
        sdrb_my_t = []
        for i in range(2):
            t_ = pp.tile([128, 1], F32, name=f"sdrbmy{i}", tag=f"sdrbmy{i}")
            nc.sync.dma_start(t_[:], sdrb_my[i * 128:(i + 1) * 128, :])
            sdrb_my_t.append(t_)
        sdrb_row_sb = pp.tile([1, D], F32, name="sdrb_row_sb")
        nc.sync.dma_start(sdrb_row_sb[:], sdr_b_row[:])
        qkb_tiles = []
        for i in range(4):
            t_ = pp.tile([128, 1], F32, name=f"qkb{i}", tag=f"qkb{i}")
            nc.sync.dma_start(t_[:], qk_bias[i * 128:(i + 1) * 128, :])
            qkb_tiles.append(t_)
        qkcs_sb = pp.tile([1, 512], F32, name="qkcs_sb")
        nc.sync.dma_start(qkcs_sb[:], qk_csum[:])
        vcs_sb = pp.tile([1, 256], F32, name="vcs_sb")
        nc.sync.dma_start(vcs_sb[:], v_csum[:])
        vbias_sb = pp.tile([128, 256], F32, name="vbias_sb")
        nc.sync.dma_start(vbias_sb[:], v_bias_bc[:])
        bout_sb = pp.tile([1, D], F32, name="bout_sb")
        nc.sync.dma_start(bout_sb[:], bout_row[:])

        x_tok = [pp.tile([128, D], F32, name=f"x_tok{i}", tag=f"x_tok{i}")
                 for i in range(2)]
        x_bf = [pp.tile([128, T], BF16, name=f"x_bf{i}", tag=f"x_bf{i}")
                for i in range(NDT)]
        # head-pair tiles: partitions 0-63 = even head, 64-127 = odd head
        qhp = [pp.tile([128, T], BF16, name=f"qhp{i}", tag=f"qhp{i}")
               for i in range(2)]
        khp = [pp.tile([128, T], BF16, name=f"khp{i}", tag=f"khp{i}")
               for i in range(2)]
        vts = [pp.tile([128, 256], BF16, name=f"vts{i}", tag=f"vts{i}")
               for i in range(8)]
        ctx_sb = [pp.tile([64, T], BF16, name=f"ctx_sb{h}", tag=f"ctx_sb{h}")
                  for h in range(HPC)]
        r_bcast = pp.tile([128, T], F32, name="r_bcast")
        negmu_row = pp.tile([1, T], F32, name="negmu_row")
        r_col = [pp.tile([128, 1], F32, name=f"r_col{i}", tag=f"r_col{i}")
                 for i in range(8)]

        # ---------------- phase 1: sdr projection ----------------
        sdrT_t = []
        for kk in range(NKK):
            t_ = sp.tile([128, T], BF16, name=f"sdrT_t{kk}", tag=f"sdrT_t{kk}")
            nc.sync.dma_start(t_[:], sdrT[kk * 128:(kk + 1) * 128, :])
            sdrT_t.append(t_)
        wsdrmy_t = []
        for kk in range(NKK):
            t_ = sp.tile([128, 256], BF16, name=f"wsdrmy{kk}", tag=f"wsdrmy{kk}")
            nc.sync.dma_start(t_[:], wsdr_my[kk * 128:(kk + 1) * 128, :])
            wsdrmy_t.append(t_)
        ag_in = dram.tile([256, T], BF16, name="ag_in")
        ag_out = dram.tile([D, T], BF16, name="ag_out")

        # own 256-feature slice of x_T, biased, -> AllGather across group
        for dt2 in range(2):
            for tt_i in range(TT):
                ps = psp.tile([128, 512], F32, name="sdr_ps", tag="mm", bufs=3)
                for kk in range(NKK):
                    nc.tensor.matmul(
                        ps[:],
                        wsdrmy_t[kk][:, dt2 * 128:(dt2 + 1) * 128],
                        sdrT_t[kk][:, tt_i * 512:(tt_i + 1) * 512],
                        start=(kk == 0), stop=(kk == NKK - 1))
                xout = sp.tile([128, 512], BF16, name="xout", tag="xout",
                               bufs=3)
                nc.scalar.activation(xout[:], ps[:], AF.Identity,
                                     bias=sdrb_my_t[dt2][:])
                nc.sync.dma_start(
                    ag_in[dt2 * 128:(dt2 + 1) * 128,
                          tt_i * 512:(tt_i + 1) * 512], xout[:])
        if single:
            for r in range(4):
                nc.sync.dma_start(ag_out[r * 256:(r + 1) * 256, :], ag_in[:])
        else:
            nc.gpsimd.collective_compute(
                "AllGather", mybir.AluOpType.bypass,
                ins=[ag_in.opt()], outs=[ag_out.opt()],
                replica_groups=[[0, 1, 2, 3], [4, 5, 6, 7]])
        for dd in range(NDT):
            nc.sync.dma_start(x_bf[dd][:], ag_out[dd * 128:(dd + 1) * 128, :])

        # LN1 stats from the gathered x
        mu_row = sp.tile([1, T], F32, name="mu_row")
        sxx_row = sp.tile([1, T], F32, name="sxx_row")
        for tt_i in range(TT):
            mu_ps = psp.tile([1, 512], F32, name="mu_ps", tag="acc", bufs=2)
            sxx_ps = psp.tile([1, 512], F32, name="sxx_ps", tag="acc", bufs=2)
            for dt_i in range(NDT):
                xsq = sp.tile([128, 512], BF16, name="xsq", tag="xsq", bufs=3)
                nc.vector.tensor_tensor(
                    xsq[:], x_bf[dt_i][:, tt_i * 512:(tt_i + 1) * 512],
                    x_bf[dt_i][:, tt_i * 512:(tt_i + 1) * 512], op=ALU.mult)
                nc.tensor.matmul(
                    mu_ps[:],
                    ones_col[:], x_bf[dt_i][:, tt_i * 512:(tt_i + 1) * 512],
                    start=(dt_i == 0), stop=(dt_i == NDT - 1))
                nc.tensor.matmul(
                    sxx_ps[:],
                    ones_col[:], xsq[:],
                    start=(dt_i == 0), stop=(dt_i == NDT - 1))
            nc.scalar.activation(mu_row[:, tt_i * 512:(tt_i + 1) * 512],
                                 mu_ps[:], AF.Copy, scale=1.0 / D)
            nc.scalar.activation(sxx_row[:, tt_i * 512:(tt_i + 1) * 512],
                                 sxx_ps[:], AF.Copy, scale=1.0 / D)

        # ---------------- LN1 stats finalize ----------------
        nc.vector.tensor_scalar(negmu_row[:], mu_row[:], -1.0, None,
                                op0=ALU.mult)
        musq = sp.tile([1, T], F32, name="musq", tag="rowtmp", bufs=2)
        nc.vector.tensor_tensor(musq[:], mu_row[:], mu_row[:], op=ALU.mult)
        var_row = sp.tile([1, T], F32, name="var_row", tag="rowtmp", bufs=2)
        nc.vector.tensor_tensor(var_row[:], sxx_row[:], musq[:],
                                op=ALU.subtract)
        lnv = sp.tile([1, T], F32, name="lnv", tag="rowtmp", bufs=2)
        nc.scalar.activation(lnv[:], var_row[:], AF.Ln, bias=EPS)
        r_row = sp.tile([1, T], F32, name="r_row", tag="rowtmp", bufs=2)
        nc.scalar.activation(r_row[:], lnv[:], AF.Exp, scale=-0.5)
        for tt_i in range(TT):
            rb_ps = psp.tile([128, 512], F32, name="rb_ps", tag="mm", bufs=3)
            nc.tensor.matmul(rb_ps[:], ones_row_f[:],
                             r_row[:, tt_i * 512:(tt_i + 1) * 512])
            nc.scalar.activation(r_bcast[:, tt_i * 512:(tt_i + 1) * 512],
                                 rb_ps[:], AF.Copy)
        nc.sync.dma_start(identf_sb[:], identf[:])
        for j in range(8):
            tp = psp.tile([128, 128], F32, name="tp", tag="quad", bufs=3)
            nc.tensor.transpose(tp[:], r_bcast[:, j * 128:(j + 1) * 128],
                                identf_sb[:])
            nc.vector.tensor_copy(r_col[j][:], tp[:, 0:1])

        # ---------------- phase 2: qkv ----------------
        wqk_t = []
        for kk in range(NDT):
            t_ = sp.tile([128, 512], BF16, name=f"wqk_t{kk}", tag=f"wqk_t{kk}")
            nc.sync.dma_start(t_[:], wqk[kk * 128:(kk + 1) * 128, :])
            wqk_t.append(t_)
        wv_t = []
        for kk in range(NDT):
            t_ = sp.tile([128, 256], BF16, name=f"wv_t{kk}", tag=f"wv_t{kk}")
            nc.sync.dma_start(t_[:], wv[kk * 128:(kk + 1) * 128, :])
            wv_t.append(t_)

        for fp in range(4):
            for tt_i in range(TT):
                ps = psp.tile([128, 512], F32, name="qk_ps", tag="mm", bufs=3)
                for kk in range(NDT):
                    nc.tensor.matmul(
                        ps[:],
                        wqk_t[kk][:, fp * 128:(fp + 1) * 128],
                        x_bf[kk][:, tt_i * 512:(tt_i + 1) * 512],
                        start=(kk == 0), stop=False)
                nc.tensor.matmul(
                    ps[:], qkcs_sb[:, fp * 128:(fp + 1) * 128],
                    negmu_row[:, tt_i * 512:(tt_i + 1) * 512],
                    start=False, stop=True)
                dst = (qhp if fp < 2 else khp)[fp % 2]
                nc.vector._custom_dve(
                    OP["ANT_TT_MULT_ADDC"],
                    out=dst[:, tt_i * 512:(tt_i + 1) * 512],
                    in0=ps[:],
                    in1=r_bcast[:, tt_i * 512:(tt_i + 1) * 512],
                    s0=qkb_tiles[fp][:])

        for vt in range(8):
            ps = psp.tile([128, 256], F32, name="v_ps", tag="mm", bufs=3)
            for kk in range(NDT):
                nc.tensor.matmul(
                    ps[:],
                    x_bf[kk][:, vt * 128:(vt + 1) * 128],
                    wv_t[kk][:],
                    start=(kk == 0), stop=False)
            nc.tensor.matmul(
                ps[:], negmu_row[:, vt * 128:(vt + 1) * 128],
                vcs_sb[:], start=False, stop=True)
            nc.vector._custom_dve(
                OP["ANT_MUL_C_ADD_T"], out=vts[vt][:], in0=ps[:],
                in1=vbias_sb[:], s0=r_col[vt][:])

        if debug:
            for tt_i in range(2):
                nc.sync.dma_start(
                    dbg["dbg_xtok"][tt_i * 128:(tt_i + 1) * 128, :],
                    x_tok[tt_i][:])
            for i in range(2):
                nc.sync.dma_start(dbg["dbg_q"][i * 128:(i + 1) * 128, :],
                                  qhp[i][:])
                nc.sync.dma_start(dbg["dbg_k"][i * 128:(i + 1) * 128, :],
                                  khp[i][:])
            for vt in range(8):
                nc.sync.dma_start(dbg["dbg_v"][vt * 128:(vt + 1) * 128, :],
                                  vts[vt][:])

        sp.release()
        ap_ = tc.alloc_tile_pool(name="attn", bufs=1)

        # ---------------- phase 3: attention ----------------
        wout_t = []
        for h in range(HPC):
            t_ = ap_.tile([64, D], BF16, name=f"wout_t{h}", tag=f"wout_t{h}")
            nc.sync.dma_start(t_[:], wout[h * 64:(h + 1) * 64, :])
            wout_t.append(t_)

        mask_t = []
        for v in range(4):
            m = pp.tile([128, 512], BF16, name=f"mask{v}", tag=f"mask{v}")
            nc.sync.dma_start(m[:], masks[v * 128:(v + 1) * 128, :])
            mask_t.append(m)
        for h in range(HPC):
            for qt in range(TT):
                nkt = 4 * (qt + 1)
                n_mac = (nkt + 3) // 4
                ctx_ps = psp.tile([64, 512], F32, name="ctx_ps", tag="acc",
                                  bufs=2)
                den_ps = psp.tile([1, 512], F32, name="den_ps", tag="acc",
                                  bufs=2)
                for mac in range(n_mac):
                    kts = list(range(mac * 4, min((mac + 1) * 4, nkt)))
                    mwd = len(kts) * 512
                    u_buf = ap_.tile([128, 2048], F32, name="u_buf",
                                     tag="u_buf", bufs=3)
                    h_buf = ap_.tile([128, 2048], F32, name="h_buf",
                                     tag="h_buf", bufs=3)
                    p_buf = ap_.tile([128, 2048], BF16, name="p_buf",
                                     tag="p_buf", bufs=3)
                    hb = (h % 2) * 64
                    for i, kt in enumerate(kts):
                        ps = psp.tile([128, 512], F32, name="s_ps", tag="mm",
                                      bufs=3)
                        nc.tensor.matmul(
                            ps[:],
                            khp[h // 2][hb:hb + 64, kt * 128:(kt + 1) * 128],
                            qhp[h // 2][hb:hb + 64, qt * 512:(qt + 1) * 512])
                        nc.scalar.activation(
                            u_buf[:, i * 512:(i + 1) * 512], ps[:],
                            AF.Relu, bias=8.0 * CLAMP)
                    nc.scalar.activation(
                        h_buf[:, 0:mwd], u_buf[:, 0:mwd], AF.Square,
                        bias=SQ_BIAS, scale=SQ_SCALE)
                    nc.vector._custom_dve(
                        H3S, out=h_buf[:, 0:mwd], in0=h_buf[:, 0:mwd],
                        in1=u_buf[:, 0:mwd],
                        s0=SQ_GAMMA, s1=HC[0], imm2=HC[1])
                    nc.vector._custom_dve(
                        OP["ANT_H3"], out=h_buf[:, 0:mwd],
                        in0=h_buf[:, 0:mwd], in1=u_buf[:, 0:mwd],
                        s0=HC[2], s1=HC[3], imm2=HC[4])
                    nc.vector._custom_dve(
                        OP["ANT_H3"], out=h_buf[:, 0:mwd],
                        in0=h_buf[:, 0:mwd], in1=u_buf[:, 0:mwd],
                        s0=HC[5], s1=HC[6], imm2=HC[7])
                    nc.scalar.activation(p_buf[:, 0:mwd], h_buf[:, 0:mwd],
                                         AF.Exp)
                    for i, kt in enumerate(kts):
                        dv = kt * 128 - qt * 512
                        if dv >= 0:
                            nc.vector.tensor_tensor(
                                p_buf[:, i * 512:(i + 1) * 512],
                                p_buf[:, i * 512:(i + 1) * 512],
                                mask_t[dv // 128][:], op=ALU.mult)
                    for i, kt in enumerate(kts):
                        first = (mac == 0 and i == 0)
                        last = (mac == n_mac - 1) and (i == len(kts) - 1)
                        nc.tensor.matmul(
                            ctx_ps[:], vts[kt][:, h * 64:(h + 1) * 64],
                            p_buf[:, i * 512:(i + 1) * 512],
                            start=first, stop=last)
                        nc.tensor.matmul(
                            den_ps[:], ones_col[:],
                            p_buf[:, i * 512:(i + 1) * 512],
                            start=first, stop=last)
                den_sb = ap_.tile([1, 512], F32, name="den_sb", tag="den_sb",
                                  bufs=2)
                nc.scalar.activation(den_sb[:], den_ps[:], AF.Copy)
                rec_sb = ap_.tile([1, 512], F32, name="rec_sb", tag="rec_sb",
                                  bufs=2)
                nc.vector.reciprocal_approx_fast(rec_sb[:], den_sb[:])
                recb_ps = psp.tile([64, 512], F32, name="recb_ps", tag="quad",
                                   bufs=3)
                nc.tensor.matmul(recb_ps[:], ones_row_f[:, 0:64], rec_sb[:])
                recb_sb = ap_.tile([64, 512], F32, name="recb_sb",
                                   tag="recb_sb", bufs=2)
                nc.scalar.activation(recb_sb[:], recb_ps[:], AF.Copy)
                nc.vector.tensor_tensor(
                    ctx_sb[h][:, qt * 512:(qt + 1) * 512],
                    ctx_ps[:], recb_sb[:], op=ALU.mult)

        if debug:
            for h in range(HPC):
                nc.sync.dma_start(dbg["dbg_ctx"][h * 64:(h + 1) * 64, :],
                                  ctx_sb[h][:])

        # ------- phase 4: out-projection + pipelined half-ReduceScatters ----
        # Token halves A (batch tokens 0-511) and B (512-1023) are reduced in
        # separate collectives so half A's residual/LN2 overlaps half B's RS.
        # Rank g owns tokens [128g,128g+128) of each half.
        b_in = [dram.tile([T // 2, D], BF16, name=f"b_in{i}") for i in range(2)]
        b_out = [dram.tile([128, D], BF16, name=f"b_out{i}") for i in range(2)]
        for half in range(2):
            for mt2 in range(4):
                mt = half * 4 + mt2
                for dc in range(2):
                    ps = psp.tile([128, 512], F32, name="op_ps", tag="mm",
                                  bufs=3)
                    for h in range(HPC):
                        nc.tensor.matmul(
                            ps[:],
                            ctx_sb[h][:, mt * 128:(mt + 1) * 128],
                            wout_t[h][:, dc * 512:(dc + 1) * 512],
                            start=(h == 0), stop=False)
                    nc.tensor.matmul(
                        ps[:], ones_row_f[:],
                        bout_sb[:, dc * 512:(dc + 1) * 512],
                        start=False, stop=True)
                    po = ap_.tile([128, 512], BF16, name="po", tag="po", bufs=2)
                    nc.scalar.activation(po[:], ps[:], AF.Copy)
                    nc.sync.dma_start(
                        b_in[half][mt2 * 128:(mt2 + 1) * 128,
                                   dc * 512:(dc + 1) * 512], po[:])
                    if debug:
                        nc.sync.dma_start(
                            dbg["dbg_attn"][mt * 128:(mt + 1) * 128,
                                            dc * 512:(dc + 1) * 512], po[:])
            if single:
                nc.sync.dma_start(b_out[half][:], b_in[half][0:128, :])
            else:
                nc.gpsimd.collective_compute(
                    "ReduceScatter", mybir.AluOpType.add,
                    ins=[b_in[half].opt()], outs=[b_out[half].opt()],
                    replica_groups=[[0, 1, 2, 3], [4, 5, 6, 7]])

        # deferred loads (low priority; only the RS-gap projection needs them)
        sdrTtok_t = []
        for kk in range(NKK):
            t_ = sp2.tile([128, TPC], BF16, name=f"sdrTtok{kk}",
                          tag=f"sdrTtok{kk}")
            nc.sync.dma_start(t_[:], sdrT_tok[kk * 128:(kk + 1) * 128, :])
            sdrTtok_t.append(t_)
        wsdr_t = []
        for kk in range(NKK):
            t_ = sp2.tile([128, D], BF16, name=f"wsdr_t{kk}", tag=f"wsdr_t{kk}")
            nc.sync.dma_start(t_[:], wsdr[kk * 128:(kk + 1) * 128, :])
            wsdr_t.append(t_)

        # x token-major fp32 (core's own tokens), fills the RS gap
        for tt_i in range(2):
            for dc in range(2):
                ps = psp.tile([128, 512], F32, name="xtok_ps", tag="mm", bufs=3)
                for kk in range(NKK):
                    nc.tensor.matmul(
                        ps[:],
                        sdrTtok_t[kk][:, tt_i * 128:(tt_i + 1) * 128],
                        wsdr_t[kk][:, dc * 512:(dc + 1) * 512],
                        start=(kk == 0), stop=False)
                nc.tensor.matmul(
                    ps[:], ones_row_f[:],
                    sdrb_row_sb[:, dc * 512:(dc + 1) * 512],
                    start=False, stop=True)
                nc.scalar.activation(
                    x_tok[tt_i][:, dc * 512:(dc + 1) * 512], ps[:], AF.Copy)

        ap_.release()
        sp2.release()
        mlp_pool = tc.alloc_tile_pool(name="mlp", bufs=1)

        # ---------------- phase 6: residual + LN2 ----------------
        x2 = [mlp_pool.tile([128, D], F32, name=f"x2_{i}", tag=f"x2_{i}")
              for i in range(2)]
        h2 = [mlp_pool.tile([128, D], BF16, name=f"h2_{i}", tag=f"h2_{i}")
              for i in range(2)]
        for tt_i in range(2):
            am = mlp_pool.tile([128, D], BF16, name="am", tag="am", bufs=2)
            nc.sync.dma_start(am[:], b_out[tt_i][:])
            nc.vector.tensor_tensor(x2[tt_i][:], x_tok[tt_i][:], am[:],
                                    op=ALU.add)
            sx = mlp_pool.tile([128, 1], F32, name="sx", tag="sx", bufs=2)
            nc.vector.reduce_sum(sx[:], x2[tt_i][:], axis=mybir.AxisListType.X)
            scratch = mlp_pool.tile([128, D], F32, name="scratch",
                                    tag="scratch", bufs=1)
            sxx2 = mlp_pool.tile([128, 1], F32, name="sxx2", tag="sxx2", bufs=2)
            nc.vector._custom_dve(
                OP["TENSOR_TENSOR_REDUCE"], out=scratch[:],
                in0=x2[tt_i][:], in1=x2[tt_i][:], s0=0.0, s1=1.0 / D,
                accum_out=sxx2[:])
            mu2 = mlp_pool.tile([128, 1], F32, name="mu2", tag="mu2", bufs=2)
            nc.vector.tensor_scalar(mu2[:], sx[:], 1.0 / D, None, op0=ALU.mult)
            mu2sq = mlp_pool.tile([128, 1], F32, name="mu2sq", tag="mu2sq",
                                  bufs=2)
            nc.vector.tensor_tensor(mu2sq[:], mu2[:], mu2[:], op=ALU.mult)
            var2 = mlp_pool.tile([128, 1], F32, name="var2", tag="var2", bufs=2)
            nc.vector.tensor_tensor(var2[:], sxx2[:], mu2sq[:],
                                    op=ALU.subtract)
            lnv2 = mlp_pool.tile([128, 1], F32, name="lnv2", tag="lnv2", bufs=2)
            nc.scalar.activation(lnv2[:], var2[:], AF.Ln, bias=EPS)
            r2 = mlp_pool.tile([128, 1], F32, name="r2", tag="r2", bufs=2)
            nc.scalar.activation(r2[:], lnv2[:], AF.Exp, scale=-0.5)
            nmr2 = mlp_pool.tile([128, 1], F32, name="nmr2", tag="nmr2", bufs=2)
            nc.vector.tensor_tensor(nmr2[:], mu2[:], r2[:], op=ALU.mult)
            nc.vector.tensor_scalar(nmr2[:], nmr2[:], -1.0, None, op0=ALU.mult)
            nc.vector.tensor_scalar(h2[tt_i][:], x2[tt_i][:], r2[:], nmr2[:],
                                    op0=ALU.mult, op1=ALU.add)

        if debug:
            for tt_i in range(2):
                nc.sync.dma_start(dbg["dbg_h2"][tt_i * 128:(tt_i + 1) * 128, :],
                                  h2[tt_i][:])

        h2T = [mlp_pool.tile([128, TPC], BF16, name=f"h2T{i}", tag=f"h2T{i}")
               for i in range(NDT)]
        nc.sync.dma_start(ident_sb[:], ident[:])
        for tt_i in range(2):
            for dd in range(NDT):
                tpb = psp.tile([128, 128], BF16, name="tpb", tag="quad", bufs=3)
                nc.tensor.transpose(
                    tpb[:], h2[tt_i][:, dd * 128:(dd + 1) * 128], ident_sb[:])
                nc.vector.tensor_copy(
                    h2T[dd][:, tt_i * 128:(tt_i + 1) * 128], tpb[:])

        # ---------------- phase 7: gate/up (feature-major) ------------
        gb_tiles, ub_tiles = [], []
        for f in range(22):
            t_ = mlp_pool.tile([128, 1], F32, name=f"gb{f}", tag=f"gb{f}")
            nc.sync.dma_start(t_[:], g_bias[f * 128:(f + 1) * 128, :])
            gb_tiles.append(t_)
            t_ = mlp_pool.tile([128, 1], F32, name=f"ub{f}", tag=f"ub{f}")
            nc.sync.dma_start(t_[:], u_bias[f * 128:(f + 1) * 128, :])
            ub_tiles.append(t_)

        suT = [mlp_pool.tile([128, TPC], BF16, name=f"suT{f}", tag=f"suT{f}")
               for f in range(22)]
        for fg in range(11):
            wgts, wuts = [], []
            for kk in range(NDT):
                wgt = mw.tile([128, 256], BF16, name="wgt", tag="wgt", bufs=10)
                nc.sync.dma_start(
                    wgt[:], wg_t[(fg * 8 + kk) * 128:(fg * 8 + kk + 1) * 128, :])
                wgts.append(wgt)
                wut = mw.tile([128, 256], BF16, name="wut", tag="wut", bufs=10)
                nc.sync.dma_start(
                    wut[:], wu_t[(fg * 8 + kk) * 128:(fg * 8 + kk + 1) * 128, :])
                wuts.append(wut)
            for f in range(2):
                fi = fg * 2 + f
                gps = psp.tile([128, TPC], F32, name="gps", tag="quad", bufs=3)
                ups = psp.tile([128, TPC], F32, name="ups", tag="quad", bufs=3)
                for kk in range(NDT):
                    nc.tensor.matmul(
                        gps[:], wgts[kk][:, f * 128:(f + 1) * 128], h2T[kk][:],
                        start=(kk == 0), stop=(kk == NDT - 1))
                    nc.tensor.matmul(
                        ups[:], wuts[kk][:, f * 128:(f + 1) * 128], h2T[kk][:],
                        start=(kk == 0), stop=(kk == NDT - 1))
                sil = mlp_pool.tile([128, TPC], BF16, name="sil", tag="sil",
                                    bufs=2)
                nc.scalar.activation(sil[:], gps[:], AF.Silu,
                                     bias=gb_tiles[fi][:])
                nc.vector._custom_dve(
                    OP["ANT_TT_ADDC_MULT"], out=suT[fi][:],
                    in0=ups[:], in1=sil[:], s0=ub_tiles[fi][:])

        if debug:
            for f in range(22):
                nc.sync.dma_start(dbg["dbg_su"][f * 128:(f + 1) * 128, :],
                                  suT[f][:])

        # ---------------- phase 8: down + residual out ----------------
        out_sb = [mlp_pool.tile([128, D], F32, name=f"out_sb{i}",
                                tag=f"out_sb{i}") for i in range(2)]
        for dc in range(2):
            pss = [psp.tile([128, 512], F32, name=f"dn_ps{i}", tag="mm",
                            bufs=3) for i in range(2)]
            for kk in range(22):
                t_ = mw.tile([128, 512], BF16, name="wdt", tag="wdt", bufs=6)
                nc.sync.dma_start(
                    t_[:], wd_t[(kk * 2 + dc) * 128:(kk * 2 + dc + 1) * 128, :])
                for tt_i in range(2):
                    nc.tensor.matmul(
                        pss[tt_i][:], suT[kk][:, tt_i * 128:(tt_i + 1) * 128],
                        t_[:], start=(kk == 0), stop=(kk == 21))
            for tt_i in range(2):
                nc.vector.tensor_tensor(
                    out_sb[tt_i][:, dc * 512:(dc + 1) * 512], pss[tt_i][:],
                    x2[tt_i][:, dc * 512:(dc + 1) * 512], op=ALU.add)
        for tt_i in range(2):
            nc.sync.dma_start(out_ap[tt_i * 128:(tt_i + 1) * 128, :],
                              out_sb[tt_i][:])

        mlp_pool.release()
        mw.release()
        dram.release()
        psp.release()
        pp.release()

    nc.compile()
    return nc


# ------------------------------------------------------------- host prep
def _prep_in_maps(inputs):
    sdr = _f32(inputs["sdr"])
    sdr_w = _f32(inputs["sdr_w"])
    sdr_b = _f32(inputs["sdr_b"])
    w_qkv = _f32(inputs["w_qkv"])
    b_qkv = _f32(inputs["b_qkv"])
    w_out = _f32(inputs["w_out"])
    b_out = _f32(inputs["b_out"])
    ln1_g, ln1_b = _f32(inputs["ln1_g"]), _f32(inputs["ln1_b"])
    ln2_g, ln2_b = _f32(inputs["ln2_g"]), _f32(inputs["ln2_b"])
    w_gate, w_up, w_down = (_f32(inputs["w_gate"]), _f32(inputs["w_up"]),
                            _f32(inputs["w_down"]))

    wqkv_f = w_qkv * ln1_g[:, None]
    bqkv_f = ln1_b @ w_qkv + b_qkv
    wg_f = w_gate * ln2_g[:, None]
    bg_f = ln2_b @ w_gate
    wu_f = w_up * ln2_g[:, None]
    bu_f = ln2_b @ w_up

    wg_p = np.zeros((D, FFN_PAD), np.float32); wg_p[:, :FFN] = wg_f
    wu_p = np.zeros((D, FFN_PAD), np.float32); wu_p[:, :FFN] = wu_f
    wd_p = np.zeros((FFN_PAD, D), np.float32); wd_p[:FFN, :] = w_down
    gb_p = np.zeros((FFN_PAD,), np.float32); gb_p[:FFN] = bg_f
    ub_p = np.zeros((FFN_PAD,), np.float32); ub_p[:FFN] = bu_f

    wg_t = _bf16(np.ascontiguousarray(
        wg_p.reshape(8, 128, 11, 256).transpose(2, 0, 1, 3)
    ).reshape(11 * 8 * 128, 256))
    wu_t = _bf16(np.ascontiguousarray(
        wu_p.reshape(8, 128, 11, 256).transpose(2, 0, 1, 3)
    ).reshape(11 * 8 * 128, 256))
    wd_t = _bf16(np.ascontiguousarray(
        wd_p.reshape(22, 128, 2, 512).transpose(0, 2, 1, 3)
    ).reshape(22 * 2 * 128, 512))

    jj = np.arange(512)[None, :]
    pp_ = np.arange(128)[:, None]
    masks = _bf16(np.concatenate(
        [(jj >= (v * 128 + pp_)).astype(np.float32) for v in range(4)], axis=0))
    ident = _bf16(np.eye(128, dtype=np.float32))
    identf = _f32(np.eye(128, dtype=np.float32))
    wsdr_bf = _bf16(sdr_w)

    sdrT_by_batch = [_bf16(sdr[b].T) for b in range(B)]
    in_maps = []
    for c in range(N_CORES):
        b, g = c // GROUP, c % GROUP
        hs = slice(g * HPC * DH, (g * HPC + HPC) * DH)
        sdrT_b = sdrT_by_batch[b]
        # core's tokens: [128g,128g+128) of each 512-token half
        sdrT_tok = np.ascontiguousarray(np.concatenate(
            [sdrT_b[:, g * 128:(g + 1) * 128],
             sdrT_b[:, 512 + g * 128:512 + (g + 1) * 128]], axis=1))
        wq_s = wqkv_f[:, 0 * D:1 * D][:, hs]
        wk_s = wqkv_f[:, 1 * D:2 * D][:, hs]
        wv_s = wqkv_f[:, 2 * D:3 * D][:, hs]
        wqk_s = _bf16(np.concatenate([wq_s, wk_s], axis=1))
        qk_b = np.concatenate([bqkv_f[0 * D:1 * D][hs], bqkv_f[1 * D:2 * D][hs]])
        qk_cs = wqk_s.astype(np.float32).sum(axis=0)[None, :]
        wv_bf = _bf16(wv_s)
        v_cs = wv_bf.astype(np.float32).sum(axis=0)[None, :]
        v_bias = bqkv_f[2 * D:3 * D][hs]
        bout_row = (b_out if g == 0 else np.zeros_like(b_out))[None, :]
        in_maps.append({
            "sdrT": sdrT_b,
            "sdrT_tok": sdrT_tok,
            "wsdr": wsdr_bf,
            "wsdr_my": np.ascontiguousarray(wsdr_bf[:, g * 256:(g + 1) * 256]),
            "sdrb_my": np.ascontiguousarray(
                sdr_b[g * 256:(g + 1) * 256][:, None].astype(np.float32)),
            "sdr_b_row": np.ascontiguousarray(sdr_b[None, :]),
            "wqk": wqk_s,
            "qk_bias": np.ascontiguousarray(qk_b[:, None]),
            "qk_csum": np.ascontiguousarray(qk_cs),
            "wv": wv_bf,
            "v_csum": np.ascontiguousarray(v_cs),
            "v_bias_bc": np.ascontiguousarray(
                np.tile(v_bias[None, :], (128, 1)).astype(np.float32)),
            "wout": _bf16(w_out[hs, :]),
            "bout_row": np.ascontiguousarray(bout_row),
            "masks": masks,
            "ident": ident,
            "identf": identf,
            "wg_t": wg_t,
            "wu_t": wu_t,
            "g_bias": np.ascontiguousarray(gb_p[:, None]),
            "u_bias": np.ascontiguousarray(ub_p[:, None]),
            "wd_t": wd_t,
        })
    return in_maps


_GRAPH_CACHE = {}


def _get_graph(debug=False):
    if debug not in _GRAPH_CACHE:
        _GRAPH_CACHE[debug] = build_graph(debug=debug)
    return _GRAPH_CACHE[debug]


def kernel(**inputs):
    nc = _get_graph(debug=False)
    in_maps = _prep_in_maps(inputs)
    res = run_bass_kernel_spmd(nc, in_maps, core_ids=list(range(N_CORES)))
    out = np.zeros((B, T, D), np.float32)
    for c in range(N_CORES):
        b, g = c // GROUP, c % GROUP
        sl = res.results[c]["out_slice"]
        out[b, g * 128:(g + 1) * 128, :] = sl[0:128]
        out[b, 512 + g * 128:512 + (g + 1) * 128, :] = sl[128:256]
    return out

